# revision 11
# baseline (speedup 1.0000x reference)
"""ContrastiveTokenLoss on Trainium2 (8 NeuronCores, Bass/Tile).

Problem (hardcoded): input [2, 2048, 32000] f32 logits, target [2, 2048] i64.
ct_len = round(2048*0.25) = 512, win = round(512*0.5) = 256,
IGNORE_INDEX = -100, PAD_ID = 0.

loss = sum_{b, i<512} valid(b,i) * log1p( sum_{j in [i-256, i), tgt[b,j]!=0}
           exp(x[b,i,tgt[b,j]] - x[b,i,tgt_safe[b,i]]) ) / max(#valid, 1)

Sharding: the 512 contrastive positions are split across the 8 cores (64 per
core per batch element; positions >= 512 are never touched).  Each core
receives its slab of logits laid out vocab-major ([32000, 128], so that the
one vocab row the loss gathers per window token is contiguous), the flattened
target window [640] covering sequence positions [64k-256, 64k+64) for both
batch elements (zero-padded on the left for cores 0..3 — pad value 0 ==
PAD_ID is masked by the same rule that masks PAD negatives), its own 128
targets, and two constant mask tables.

On-device per core: 5 indirect DMAs gather the 640 window rows (512 B
contiguous each, ~320 KB instead of the 16 MB slab), PE transposes the
gathered [128,128] tiles back to (position-partition, window-free) layout,
then DVE/ACT compute the masked exp / log1p reduction per row and one
[128,2]x[128,1] matmul reduces loss / valid-count over partitions.  Each
core returns [loss_sum, valid_count]; the host sums 8 partials and divides.
"""

import numpy as np
from contextlib import ExitStack

import concourse.bass as bass
import concourse.bacc as bacc
import concourse.mybir as mybir
import concourse.tile as tile
from concourse.bass_utils import run_bass_kernel_spmd
from concourse.masks import make_identity

B, T, V = 2, 2048, 32000
CT = 512
WIN = 256
IGNORE_INDEX = -100
PAD_ID = 0
NCORES = 8
CI = CT // NCORES          # 64 positions per core per batch
W = WIN + CI               # 320 window positions per core per batch
NW = B * W                 # 640 window rows gathered per core
P = B * CI                 # 128 partition rows = (batch, local position)
NCH = NW // P              # 5 gather chunks of 128 rows
F32 = mybir.dt.float32
I32 = mybir.dt.int32

_CACHE = {}


def _build():
    nc = bacc.Bacc("TRN2", target_bir_lowering=False)
    xt = nc.dram_tensor("xt", [V, P], F32, kind="ExternalInput")
    tw = nc.dram_tensor("tw", [1, NW], I32, kind="ExternalInput")
    to = nc.dram_tensor("to", [P, 1], I32, kind="ExternalInput")
    band = nc.dram_tensor("band", [P, NW], F32, kind="ExternalInput")
    diag = nc.dram_tensor("diag", [P, NW], F32, kind="ExternalInput")
    out = nc.dram_tensor("out", [2, 1], F32, kind="ExternalOutput")

    with ExitStack() as ctx:
        tc = ctx.enter_context(tile.TileContext(nc))
        sb = ctx.enter_context(tc.tile_pool(name="sb", bufs=1))
        ps = ctx.enter_context(tc.tile_pool(name="ps", bufs=1, space="PSUM"))

        band_sb = sb.tile([P, NW], F32)
        nc.sync.dma_start(band_sb[:], band[:])
        diag_sb = sb.tile([P, NW], F32)
        nc.sync.dma_start(diag_sb[:], diag[:])
        to_sb = sb.tile([P, 1], I32)
        nc.sync.dma_start(to_sb[:], to[:])
        tw_row = sb.tile([1, NW], I32)
        nc.sync.dma_start(tw_row[:], tw[:])

        ident = sb.tile([P, P], F32)
        make_identity(nc, ident[:])

        # tj[p, f] = tw[f]; cross-batch columns are zeroed by `band` later
        tj = sb.tile([P, NW], I32)
        nc.gpsimd.partition_broadcast(tj[:], tw_row[:])

        # negative-validity mask: band(b, i_local, j_local) & (tgt[b, j] != PAD)
        m = sb.tile([P, NW], F32)
        nc.vector.tensor_scalar(m[:], tj[:], PAD_ID, None, mybir.AluOpType.not_equal)
        nc.vector.tensor_mul(m[:], m[:], band_sb[:])

        # gather the 640 window rows (contiguous [1, P] rows of xt), then
        # transpose back so partitions index (batch, position)
        g = sb.tile([P, NW], F32)
        for c in range(NCH):
            offs_raw = sb.tile([P, 1], I32, tag=f"offsr{c}")
            nc.sync.dma_start(
                offs_raw[:],
                tw[0:1, c * P : (c + 1) * P].rearrange("a b -> (a b) ()"),
            )
            offs = sb.tile([P, 1], I32, tag=f"offs{c}")
            nc.vector.tensor_scalar(
                offs[:], offs_raw[:], 0, None, mybir.AluOpType.max
            )
            gt = sb.tile([P, P], F32, tag=f"gt{c}")
            nc.gpsimd.indirect_dma_start(
                out=gt[:],
                out_offset=None,
                in_=xt[:],
                in_offset=bass.IndirectOffsetOnAxis(ap=offs[:], axis=0),
            )
            pt = ps.tile([P, P], F32, tag=f"pt{c}", space="PSUM")
            nc.tensor.transpose(out=pt[:], in_=gt[:], identity=ident[:])
            nc.vector.tensor_copy(g[:, c * P : (c + 1) * P], pt[:])

        # pos[p] = g[p, (p // CI) * W + WIN + (p % CI)] via diag mask; negated
        # so it can be the bias of the fused exp
        gd = sb.tile([P, NW], F32)
        nc.vector.tensor_mul(gd[:], g[:], diag_sb[:])
        npos = sb.tile([P, 1], F32)
        nc.vector.reduce_sum(npos[:], gd[:], axis=mybir.AxisListType.X, negate=True)

        # e = exp(g - pos) * mask ; r[p] = sum_f e
        e = sb.tile([P, NW], F32)
        nc.scalar.activation(
            e[:], g[:], mybir.ActivationFunctionType.Exp, bias=npos[:], scale=1.0
        )
        nc.vector.tensor_mul(e[:], e[:], m[:])
        r = sb.tile([P, 1], F32)
        nc.vector.reduce_sum(r[:], e[:], axis=mybir.AxisListType.X)

        # lv[:, 0] = ln(1 + r) * valid ; lv[:, 1] = valid
        lv = sb.tile([P, 2], F32)
        nc.scalar.activation(
            lv[:, 0:1], r[:], mybir.ActivationFunctionType.Ln, bias=1.0, scale=1.0
        )
        nc.vector.tensor_scalar(
            lv[:, 1:2], to_sb[:], IGNORE_INDEX, None, mybir.AluOpType.not_equal
        )
        nc.vector.tensor_mul(lv[:, 0:1], lv[:, 0:1], lv[:, 1:2])

        # partition reduction: out[2, 1] = lv.T @ ones
        ones = sb.tile([P, 1], F32)
        nc.vector.memset(ones[:], 1.0)
        acc = ps.tile([2, 1], F32, space="PSUM")
        nc.tensor.matmul(out=acc[:], lhsT=lv[:], rhs=ones[:], start=True, stop=True)
        res = sb.tile([2, 1], F32)
        nc.vector.tensor_copy(res[:], acc[:])
        nc.sync.dma_start(out[:], res[:])
    nc.compile()
    return nc


def _get_nc():
    if "nc" not in _CACHE:
        _CACHE["nc"] = _build()
    return _CACHE["nc"]


def _consts():
    if "consts" not in _CACHE:
        p = np.arange(P, dtype=np.int64)
        il = (p % CI)[:, None]
        bp = (p // CI)[:, None]
        f = np.arange(NW, dtype=np.int64)[None, :]
        jl = f % W
        bf = f // W
        band = ((bf == bp) & (jl >= il) & (jl < il + WIN)).astype(np.float32)
        diag = ((bf == bp) & (jl == il + WIN)).astype(np.float32)
        _CACHE["consts"] = (np.ascontiguousarray(band), np.ascontiguousarray(diag))
    return _CACHE["consts"]


def kernel(input, target, _trace=False):
    input = np.asarray(input, dtype=np.float32)
    target = np.asarray(target)
    band, diag = _consts()
    t32 = target[:, :CT].astype(np.int32)

    in_maps = []
    for k in range(NCORES):
        s = k * CI
        lo = s - WIN
        if lo >= 0:
            twk = t32[:, lo : s + CI]
        else:
            twk = np.concatenate(
                [np.zeros((B, -lo), np.int32), t32[:, : s + CI]], axis=1
            )
        in_maps.append(
            {
                "xt": np.ascontiguousarray(
                    input[:, s : s + CI, :].reshape(P, V).T
                ),
                "tw": np.ascontiguousarray(twk).reshape(1, NW),
                "to": np.ascontiguousarray(t32[:, s : s + CI]).reshape(P, 1),
                "band": band,
                "diag": diag,
            }
        )

    nc = _get_nc()
    br = run_bass_kernel_spmd(
        nc, in_maps, core_ids=list(range(NCORES)), trace=_trace
    )
    rs = np.stack([r["out"] for r in br.results])  # [8, 2, 1]
    loss_sum = rs[:, 0, 0].astype(np.float64).sum()
    cnt = rs[:, 1, 0].astype(np.float64).sum()
    kernel.last_results = br
    return np.asarray(np.float32(loss_sum / max(cnt, 1.0)))
